# revision 15
# baseline (speedup 1.0000x reference)
"""Trainium2 Bass kernel for BiaffineSpanHead.

Math (per batch b):
  Hs = x @ Ws_w.T + Ws_b            [S, H]
  He = x @ We_w.T + We_b            [S, H]
  biaff[s,e,c] = sum_{h,g} Hs[s,h] U[h,c,g] He[e,g]
  out[s,e,c] = biaff + Ls[s,c] + Le[e,c] + W_b[c]
      Ls = Hs @ W_w[:, :H].T,  Le = He @ W_w[:, H:].T

Sharding: data-parallel over batch B=8 across 8 cores (one batch each).
Per-core device layout: out[c, s, e] (contiguous [32, 512, 512] f32);
host transposes back to [B, S, E, C].

Device dataflow per core (P=128 partitions):
  xT [D,S] -> 6 SBUF tiles [128, 512] bf16
  HsT/HeT [H,S]: 2 psum m-tiles, 6 k-tiles each; ACT copy + per-partition
      bias -> bf16 SBUF tiles [128, 512]
  Ls   [s,c]: 4 s-tiles [128, 32] f32 (lhsT=HsT slice, rhs=W_s.T)
  LeWb [c,e]: [32, 512] bf16 (lhsT=W_e.T, rhs=HeT) + W_b bias
  per label c in 0..31:
    T_c[g,s]  = U[:,c,:].T-contracted with HsT   (2 m-tiles x 2 k-tiles)
    bc[p,e]   = indicator-matmul replicate of LeWb[c,:] across partitions
    biaff s-tile [128(s), 512(e)] = T_c.T @ HeT  (2 k-tiles)
    out tile  = DVE (psum + Ls[s,c]) + bc        (scalar_tensor_tensor)
    DMA -> out[c, s-tile, :]
"""

import sys

if "/opt/trn_rl_repo" not in sys.path:
    sys.path.insert(0, "/opt/trn_rl_repo")

import numpy as np
import ml_dtypes

BF16 = ml_dtypes.bfloat16

B, S, D = 8, 512, 768
H, C = 256, 32
P = 128
KD = D // P   # 6 k-tiles for the D contraction
KH = H // P   # 2 k-tiles for the H/G contraction
NS = S // P   # 4 s-tiles
N_CORES = 8

_CACHE: dict = {}


def _build_nc():
    from contextlib import ExitStack

    import concourse.tile as tile
    from concourse import bacc, mybir

    f32 = mybir.dt.float32
    bf16 = mybir.dt.bfloat16
    ID = mybir.ActivationFunctionType.Identity
    CP = mybir.ActivationFunctionType.Copy
    ADD = mybir.AluOpType.add

    nc = bacc.Bacc("TRN2", target_bir_lowering=False, debug=False,
                   num_devices=N_CORES)

    xT_d = nc.dram_tensor("xT", [D, S], bf16, kind="ExternalInput").ap()
    wswT_d = nc.dram_tensor("wswT", [D, H], bf16, kind="ExternalInput").ap()
    wewT_d = nc.dram_tensor("wewT", [D, H], bf16, kind="ExternalInput").ap()
    u_d = nc.dram_tensor("u", [H, C * H], bf16, kind="ExternalInput").ap()
    wsb_d = nc.dram_tensor("wsb", [KH, P, 1], f32, kind="ExternalInput").ap()
    web_d = nc.dram_tensor("web", [KH, P, 1], f32, kind="ExternalInput").ap()
    wstl_d = nc.dram_tensor("wstl", [H, C], bf16, kind="ExternalInput").ap()
    wetl_d = nc.dram_tensor("wetl", [H, C], bf16, kind="ExternalInput").ap()
    wb_d = nc.dram_tensor("wb", [C, 1], f32, kind="ExternalInput").ap()
    ind_d = nc.dram_tensor("ind", [C, C * P], bf16, kind="ExternalInput").ap()
    out_d = nc.dram_tensor("out", [C, S, S], f32, kind="ExternalOutput").ap()

    with tile.TileContext(nc) as tc, ExitStack() as ctx:
        consts = ctx.enter_context(tc.tile_pool(name="consts", bufs=1))
        psum = ctx.enter_context(tc.tile_pool(name="psum", bufs=1, space="PSUM"))
        tcp = ctx.enter_context(tc.tile_pool(name="tcp", bufs=1))
        bcp = ctx.enter_context(tc.tile_pool(name="bcp", bufs=2))
        outp = ctx.enter_context(tc.tile_pool(name="outp", bufs=6))

        # ---- load constants / inputs into SBUF ----
        xT_r = xT_d.rearrange("(k p) s -> k p s", p=P)
        wswT_r = wswT_d.rearrange("(k p) h -> k p h", p=P)
        wewT_r = wewT_d.rearrange("(k p) h -> k p h", p=P)
        u_r = u_d.rearrange("(k p) f -> k p f", p=P)
        wstl_r = wstl_d.rearrange("(k p) c -> k p c", p=P)
        wetl_r = wetl_d.rearrange("(k p) c -> k p c", p=P)

        # Small weights first (cheap to issue, needed right after the
        # projections), then the projection-critical big tiles split
        # across both DMA queues so neither serializes the start.
        xt, wsw, wew, usb, wstl, wetl = [], [], [], [], [], []
        wsb_t, web_t = [], []
        for m in range(KH):
            t = consts.tile([P, 1], f32, tag=f"wsb{m}", name=f"wsb{m}")
            nc.sync.dma_start(t, wsb_d[m])
            wsb_t.append(t)
            t = consts.tile([P, 1], f32, tag=f"web{m}", name=f"web{m}")
            nc.sync.dma_start(t, web_d[m])
            web_t.append(t)
        wb_t = consts.tile([C, 1], f32, tag="wb", name="wb")
        nc.sync.dma_start(wb_t, wb_d)
        for k in range(KH):
            t = consts.tile([P, C], bf16, tag=f"wstl{k}", name=f"wstl{k}")
            nc.sync.dma_start(t, wstl_r[k])
            wstl.append(t)
            t = consts.tile([P, C], bf16, tag=f"wetl{k}", name=f"wetl{k}")
            nc.sync.dma_start(t, wetl_r[k])
            wetl.append(t)
        for k in range(KD):
            q = nc.sync if k % 2 == 0 else nc.gpsimd
            t = consts.tile([P, S], bf16, tag=f"xt{k}", name=f"xt{k}")
            q.dma_start(t, xT_r[k])
            xt.append(t)
            t = consts.tile([P, H], bf16, tag=f"wsw{k}", name=f"wsw{k}")
            q.dma_start(t, wswT_r[k])
            wsw.append(t)
            t = consts.tile([P, H], bf16, tag=f"wew{k}", name=f"wew{k}")
            q.dma_start(t, wewT_r[k])
            wew.append(t)
        # U streams on the gpsimd queue, column-chunked so early labels can
        # start before the whole 4 MB lands (Tile tracks subtile deps).
        ind_t = consts.tile([C, C * P], bf16, tag="ind", name="ind")
        nc.gpsimd.dma_start(ind_t, ind_d)
        UCH = 8
        for k in range(KH):
            t = consts.tile([P, C * H], bf16, tag=f"u{k}", name=f"u{k}")
            usb.append(t)
        for j in range(UCH):
            w = C * H // UCH
            for k in range(KH):
                nc.gpsimd.dma_start(usb[k][:, j * w:(j + 1) * w],
                                    u_r[k][:, j * w:(j + 1) * w])
        # ---- stage A: projections HsT / HeT  [H, S] as 2 bf16 tiles ----
        hst, het = [], []
        for nm, wt, bias, dst in (("hs", wsw, wsb_t, hst),
                                  ("he", wew, web_t, het)):
            for m in range(KH):
                ps = psum.tile([P, S], f32, tag="psO", bufs=4,
                               name=f"ps_{nm}{m}")
                for k in range(KD):
                    nc.tensor.matmul(ps, lhsT=wt[k][:, m * P:(m + 1) * P],
                                     rhs=xt[k], start=(k == 0),
                                     stop=(k == KD - 1))
                ht = consts.tile([P, S], bf16, tag=f"{nm}t{m}",
                                 name=f"{nm}t{m}")
                nc.scalar.activation(ht, ps, ID, bias=bias[m])
                dst.append(ht)

        # ---- Ls s-tiles [128, 32] f32 ----
        ls_t = []
        for st in range(NS):
            ps = psum.tile([P, C], f32, tag="psBC", bufs=1, name=f"ps_ls{st}")
            for k in range(KH):
                nc.tensor.matmul(ps, lhsT=hst[k][:, st * P:(st + 1) * P],
                                 rhs=wstl[k], start=(k == 0),
                                 stop=(k == KH - 1))
            lt = consts.tile([P, C], f32, tag=f"ls{st}", name=f"ls{st}")
            nc.vector.tensor_copy(lt, ps)
            ls_t.append(lt)

        # ---- LeWb [32, 512] bf16 (Le.T + W_b bias) ----
        ps = psum.tile([C, S], f32, tag="psBC", bufs=1, name="ps_le")
        for k in range(KH):
            nc.tensor.matmul(ps, lhsT=wetl[k], rhs=het[k], start=(k == 0),
                             stop=(k == KH - 1))
        lewb = consts.tile([C, S], bf16, tag="lewb", name="lewb")
        nc.scalar.activation(lewb, ps, ID, bias=wb_t)

        # ---- stage B: per-label pipeline ----
        for c in range(C):
            # T_c[g, s] = sum_h U[h, c, g] HsT[h, s]   (2 g-tiles)
            tc_sb = []
            for mg in range(KH):
                pst = psum.tile([P, S], f32, tag="psT", bufs=3,
                                name=f"ps_t{c}_{mg}")
                off = c * H + mg * P
                for kh in range(KH):
                    nc.tensor.matmul(pst, lhsT=usb[kh][:, off:off + P],
                                     rhs=hst[kh], start=(kh == 0),
                                     stop=(kh == KH - 1))
                t = tcp.tile([P, S], bf16, tag=f"tc{mg}", bufs=2,
                             name=f"tc{c}_{mg}")
                nc.scalar.activation(t, pst, CP)
                tc_sb.append(t)

            # The broadcast lin row (LeWb[c, :] replicated) is materialized
            # ONCE into s-tile 0's PSUM bank via the indicator matmul. ACT
            # snapshots it to SBUF (bc) for the DVE-evicted s-tiles 1-3;
            # st 0's biaff then accumulates on top of it in the same bank
            # and is evicted by ACT with the Ls bias. One [128, 2048] out
            # tile -> single DMA per c.
            ot = outp.tile([P, NS * S], f32, tag="ot", bufs=4,
                           name=f"ot{c}")
            po0 = psum.tile([P, S], f32, tag="psO", bufs=4, name=f"ps_o{c}_0")
            nc.tensor.matmul(po0, lhsT=ind_t[:, c * P:(c + 1) * P],
                             rhs=lewb, start=True, stop=False)
            bc = bcp.tile([P, S], f32, tag="bc", bufs=2, name=f"bc{c}")
            nc.scalar.activation(bc, po0, CP)

            for st in range(1, NS):
                po = psum.tile([P, S], f32, tag="psO", bufs=4,
                               name=f"ps_o{c}_{st}")
                for kg in range(KH):
                    nc.tensor.matmul(po,
                                     lhsT=tc_sb[kg][:, st * P:(st + 1) * P],
                                     rhs=het[kg], start=(kg == 0),
                                     stop=(kg == KH - 1))
                nc.vector.scalar_tensor_tensor(ot[:, st * S:(st + 1) * S],
                                               po, ls_t[st][:, c:c + 1], bc,
                                               op0=ADD, op1=ADD)
            # st 0 last: accumulate biaff onto the broadcast row already in
            # the bank (after ACT's snapshot read).
            for kg in range(KH):
                nc.tensor.matmul(po0, lhsT=tc_sb[kg][:, 0:P], rhs=het[kg],
                                 start=False, stop=(kg == KH - 1))
            nc.scalar.activation(ot[:, 0:S], po0, ID,
                                 bias=ls_t[0][:, c:c + 1])
            eng = nc.sync if c % 2 else nc.gpsimd
            dst = out_d[c].rearrange("(st p) e -> p st e", p=P)
            eng.dma_start(dst, ot.rearrange("p (st e) -> p st e", e=S))

    nc.compile()
    return nc


def _host_prep(seq_feats, U, W_w, W_b, Ws_w, Ws_b, We_w, We_b):
    """Build the per-core input maps (host-side layout prep, all small
    except seq_feats transpose)."""
    ind = np.zeros((C, C * P), dtype=BF16)
    for k in range(C):
        ind[k, k * P:(k + 1) * P] = 1.0

    common = {
        "wswT": np.ascontiguousarray(Ws_w.T).astype(BF16),
        "wewT": np.ascontiguousarray(We_w.T).astype(BF16),
        "u": np.ascontiguousarray(U.reshape(H, C * H)).astype(BF16),
        "wsb": np.ascontiguousarray(Ws_b.reshape(KH, P, 1)).astype(np.float32),
        "web": np.ascontiguousarray(We_b.reshape(KH, P, 1)).astype(np.float32),
        "wstl": np.ascontiguousarray(W_w[:, :H].T).astype(BF16),
        "wetl": np.ascontiguousarray(W_w[:, H:].T).astype(BF16),
        "wb": np.ascontiguousarray(W_b.reshape(C, 1)).astype(np.float32),
        "ind": ind,
    }
    in_maps = []
    for b in range(B):
        m = dict(common)
        m["xT"] = np.ascontiguousarray(seq_feats[b].T).astype(BF16)
        in_maps.append(m)
    return in_maps


def kernel(seq_feats, U, W_w, W_b, Ws_w, Ws_b, We_w, We_b):
    from concourse.bass_utils import run_bass_kernel_spmd

    seq_feats = np.asarray(seq_feats, dtype=np.float32)
    U = np.asarray(U, dtype=np.float32)
    W_w = np.asarray(W_w, dtype=np.float32)
    W_b = np.asarray(W_b, dtype=np.float32)
    Ws_w = np.asarray(Ws_w, dtype=np.float32)
    Ws_b = np.asarray(Ws_b, dtype=np.float32)
    We_w = np.asarray(We_w, dtype=np.float32)
    We_b = np.asarray(We_b, dtype=np.float32)

    if "nc" not in _CACHE:
        _CACHE["nc"] = _build_nc()
    nc = _CACHE["nc"]

    in_maps = _host_prep(seq_feats, U, W_w, W_b, Ws_w, Ws_b, We_w, We_b)
    res = run_bass_kernel_spmd(nc, in_maps, core_ids=list(range(N_CORES)))
    _CACHE["last_result"] = res

    out = np.stack([res.results[b]["out"] for b in range(B)])  # [B, C, S, S]
    return np.ascontiguousarray(out.transpose(0, 2, 3, 1)).astype(np.float32)


# revision 16
# speedup vs baseline: 1.0486x; 1.0486x over previous
"""Trainium2 Bass kernel for BiaffineSpanHead.

Math (per batch b):
  Hs = x @ Ws_w.T + Ws_b            [S, H]
  He = x @ We_w.T + We_b            [S, H]
  biaff[s,e,c] = sum_{h,g} Hs[s,h] U[h,c,g] He[e,g]
  out[s,e,c] = biaff + Ls[s,c] + Le[e,c] + W_b[c]
      Ls = Hs @ W_w[:, :H].T,  Le = He @ W_w[:, H:].T

Sharding: data-parallel over batch B=8 across 8 cores (one batch each).
Per-core device layout: out[c, s, e] (contiguous [32, 512, 512] f32);
host transposes back to [B, S, E, C].

Device dataflow per core (P=128 partitions):
  xT [D,S] -> 6 SBUF tiles [128, 512] bf16
  HsT/HeT [H,S]: 2 psum m-tiles, 6 k-tiles each; ACT copy + per-partition
      bias -> bf16 SBUF tiles [128, 512]
  Ls   [s,c]: 4 s-tiles [128, 32] f32 (lhsT=HsT slice, rhs=W_s.T)
  LeWb [c,e]: [32, 512] bf16 (lhsT=W_e.T, rhs=HeT) + W_b bias
  per label c in 0..31:
    T_c[g,s]  = U[:,c,:].T-contracted with HsT   (2 m-tiles x 2 k-tiles)
    bc[p,e]   = indicator-matmul replicate of LeWb[c,:] across partitions
    biaff s-tile [128(s), 512(e)] = T_c.T @ HeT  (2 k-tiles)
    out tile  = DVE (psum + Ls[s,c]) + bc        (scalar_tensor_tensor)
    DMA -> out[c, s-tile, :]
"""

import sys

if "/opt/trn_rl_repo" not in sys.path:
    sys.path.insert(0, "/opt/trn_rl_repo")

import numpy as np
import ml_dtypes

BF16 = ml_dtypes.bfloat16

B, S, D = 8, 512, 768
H, C = 256, 32
P = 128
KD = D // P   # 6 k-tiles for the D contraction
KH = H // P   # 2 k-tiles for the H/G contraction
NS = S // P   # 4 s-tiles
N_CORES = 8

_CACHE: dict = {}


def _build_nc():
    from contextlib import ExitStack

    import concourse.tile as tile
    from concourse import bacc, mybir

    f32 = mybir.dt.float32
    bf16 = mybir.dt.bfloat16
    ID = mybir.ActivationFunctionType.Identity
    CP = mybir.ActivationFunctionType.Copy
    ADD = mybir.AluOpType.add

    nc = bacc.Bacc("TRN2", target_bir_lowering=False, debug=False,
                   num_devices=N_CORES)

    xT_d = nc.dram_tensor("xT", [D, S], bf16, kind="ExternalInput").ap()
    wswT_d = nc.dram_tensor("wswT", [D, H], bf16, kind="ExternalInput").ap()
    wewT_d = nc.dram_tensor("wewT", [D, H], bf16, kind="ExternalInput").ap()
    u_d = nc.dram_tensor("u", [H, C * H], bf16, kind="ExternalInput").ap()
    wsb_d = nc.dram_tensor("wsb", [KH, P, 1], f32, kind="ExternalInput").ap()
    web_d = nc.dram_tensor("web", [KH, P, 1], f32, kind="ExternalInput").ap()
    wstl_d = nc.dram_tensor("wstl", [H, C], bf16, kind="ExternalInput").ap()
    wetl_d = nc.dram_tensor("wetl", [H, C], bf16, kind="ExternalInput").ap()
    wb_d = nc.dram_tensor("wb", [C, 1], f32, kind="ExternalInput").ap()
    ind_d = nc.dram_tensor("ind", [C, C * P], bf16, kind="ExternalInput").ap()
    out_d = nc.dram_tensor("out", [C, S, S], f32, kind="ExternalOutput").ap()

    with tile.TileContext(nc) as tc, ExitStack() as ctx:
        consts = ctx.enter_context(tc.tile_pool(name="consts", bufs=1))
        psum = ctx.enter_context(tc.tile_pool(name="psum", bufs=1, space="PSUM"))
        tcp = ctx.enter_context(tc.tile_pool(name="tcp", bufs=1))
        bcp = ctx.enter_context(tc.tile_pool(name="bcp", bufs=2))
        outp = ctx.enter_context(tc.tile_pool(name="outp", bufs=6))

        # ---- load constants / inputs into SBUF ----
        xT_r = xT_d.rearrange("(k p) s -> k p s", p=P)
        wswT_r = wswT_d.rearrange("(k p) h -> k p h", p=P)
        wewT_r = wewT_d.rearrange("(k p) h -> k p h", p=P)
        u_r = u_d.rearrange("(k p) f -> k p f", p=P)
        wstl_r = wstl_d.rearrange("(k p) c -> k p c", p=P)
        wetl_r = wetl_d.rearrange("(k p) c -> k p c", p=P)

        # Projection-critical big tiles split across both DMA queues so
        # neither serializes the start; small weights follow on sync
        # (needed only after the projections, ~16 us in).
        xt, wsw, wew, usb, wstl, wetl = [], [], [], [], [], []
        for k in range(KD):
            q = nc.sync if k % 2 == 0 else nc.gpsimd
            t = consts.tile([P, S], bf16, tag=f"xt{k}", name=f"xt{k}")
            q.dma_start(t, xT_r[k])
            xt.append(t)
            t = consts.tile([P, H], bf16, tag=f"wsw{k}", name=f"wsw{k}")
            q.dma_start(t, wswT_r[k])
            wsw.append(t)
            t = consts.tile([P, H], bf16, tag=f"wew{k}", name=f"wew{k}")
            q.dma_start(t, wewT_r[k])
            wew.append(t)
        wsb_t, web_t = [], []
        for m in range(KH):
            t = consts.tile([P, 1], f32, tag=f"wsb{m}", name=f"wsb{m}")
            nc.sync.dma_start(t, wsb_d[m])
            wsb_t.append(t)
            t = consts.tile([P, 1], f32, tag=f"web{m}", name=f"web{m}")
            nc.sync.dma_start(t, web_d[m])
            web_t.append(t)
        wb_t = consts.tile([C, 1], f32, tag="wb", name="wb")
        nc.sync.dma_start(wb_t, wb_d)
        for k in range(KH):
            t = consts.tile([P, C], bf16, tag=f"wstl{k}", name=f"wstl{k}")
            nc.sync.dma_start(t, wstl_r[k])
            wstl.append(t)
            t = consts.tile([P, C], bf16, tag=f"wetl{k}", name=f"wetl{k}")
            nc.sync.dma_start(t, wetl_r[k])
            wetl.append(t)
        # U streams on the gpsimd queue, column-chunked so early labels can
        # start before the whole 4 MB lands (Tile tracks subtile deps).
        ind_t = consts.tile([C, C * P], bf16, tag="ind", name="ind")
        nc.gpsimd.dma_start(ind_t, ind_d)
        UCH = 8
        for k in range(KH):
            t = consts.tile([P, C * H], bf16, tag=f"u{k}", name=f"u{k}")
            usb.append(t)
        for j in range(UCH):
            w = C * H // UCH
            for k in range(KH):
                nc.gpsimd.dma_start(usb[k][:, j * w:(j + 1) * w],
                                    u_r[k][:, j * w:(j + 1) * w])
        # ---- stage A: projections HsT / HeT  [H, S] as 2 bf16 tiles ----
        hst, het = [], []
        for nm, wt, bias, dst in (("hs", wsw, wsb_t, hst),
                                  ("he", wew, web_t, het)):
            for m in range(KH):
                ps = psum.tile([P, S], f32, tag="psO", bufs=4,
                               name=f"ps_{nm}{m}")
                for k in range(KD):
                    nc.tensor.matmul(ps, lhsT=wt[k][:, m * P:(m + 1) * P],
                                     rhs=xt[k], start=(k == 0),
                                     stop=(k == KD - 1))
                ht = consts.tile([P, S], bf16, tag=f"{nm}t{m}",
                                 name=f"{nm}t{m}")
                nc.scalar.activation(ht, ps, ID, bias=bias[m])
                dst.append(ht)

        # ---- Ls s-tiles [128, 32] f32 ----
        ls_t = []
        for st in range(NS):
            ps = psum.tile([P, C], f32, tag="psBC", bufs=1, name=f"ps_ls{st}")
            for k in range(KH):
                nc.tensor.matmul(ps, lhsT=hst[k][:, st * P:(st + 1) * P],
                                 rhs=wstl[k], start=(k == 0),
                                 stop=(k == KH - 1))
            lt = consts.tile([P, C], f32, tag=f"ls{st}", name=f"ls{st}")
            nc.vector.tensor_copy(lt, ps)
            ls_t.append(lt)

        # ---- LeWb [32, 512] bf16 (Le.T + W_b bias) ----
        ps = psum.tile([C, S], f32, tag="psBC", bufs=1, name="ps_le")
        for k in range(KH):
            nc.tensor.matmul(ps, lhsT=wetl[k], rhs=het[k], start=(k == 0),
                             stop=(k == KH - 1))
        lewb = consts.tile([C, S], bf16, tag="lewb", name="lewb")
        nc.scalar.activation(lewb, ps, ID, bias=wb_t)

        # ---- stage B: per-label pipeline ----
        for c in range(C):
            # T_c[g, s] = sum_h U[h, c, g] HsT[h, s]   (2 g-tiles)
            tc_sb = []
            for mg in range(KH):
                pst = psum.tile([P, S], f32, tag="psT", bufs=3,
                                name=f"ps_t{c}_{mg}")
                off = c * H + mg * P
                for kh in range(KH):
                    nc.tensor.matmul(pst, lhsT=usb[kh][:, off:off + P],
                                     rhs=hst[kh], start=(kh == 0),
                                     stop=(kh == KH - 1))
                t = tcp.tile([P, S], bf16, tag=f"tc{mg}", bufs=2,
                             name=f"tc{c}_{mg}")
                nc.scalar.activation(t, pst, CP)
                tc_sb.append(t)

            # The broadcast lin row (LeWb[c, :] replicated) is materialized
            # ONCE into s-tile 0's PSUM bank via the indicator matmul. ACT
            # snapshots it to SBUF (bc) for the DVE-evicted s-tiles 1-3;
            # st 0's biaff then accumulates on top of it in the same bank
            # and is evicted by ACT with the Ls bias. One [128, 2048] out
            # tile -> single DMA per c.
            ot = outp.tile([P, NS * S], f32, tag="ot", bufs=4,
                           name=f"ot{c}")
            po0 = psum.tile([P, S], f32, tag="psO", bufs=4, name=f"ps_o{c}_0")
            nc.tensor.matmul(po0, lhsT=ind_t[:, c * P:(c + 1) * P],
                             rhs=lewb, start=True, stop=False)
            bc = bcp.tile([P, S], f32, tag="bc", bufs=2, name=f"bc{c}")
            nc.scalar.activation(bc, po0, CP)

            for st in range(1, NS):
                po = psum.tile([P, S], f32, tag="psO", bufs=4,
                               name=f"ps_o{c}_{st}")
                for kg in range(KH):
                    nc.tensor.matmul(po,
                                     lhsT=tc_sb[kg][:, st * P:(st + 1) * P],
                                     rhs=het[kg], start=(kg == 0),
                                     stop=(kg == KH - 1))
                nc.vector.scalar_tensor_tensor(ot[:, st * S:(st + 1) * S],
                                               po, ls_t[st][:, c:c + 1], bc,
                                               op0=ADD, op1=ADD)
            # st 0 last: accumulate biaff onto the broadcast row already in
            # the bank (after ACT's snapshot read).
            for kg in range(KH):
                nc.tensor.matmul(po0, lhsT=tc_sb[kg][:, 0:P], rhs=het[kg],
                                 start=False, stop=(kg == KH - 1))
            nc.scalar.activation(ot[:, 0:S], po0, ID,
                                 bias=ls_t[0][:, c:c + 1])
            eng = nc.sync if c % 2 else nc.gpsimd
            dst = out_d[c].rearrange("(st p) e -> p st e", p=P)
            eng.dma_start(dst, ot.rearrange("p (st e) -> p st e", e=S))

    nc.compile()
    return nc


def _host_prep(seq_feats, U, W_w, W_b, Ws_w, Ws_b, We_w, We_b):
    """Build the per-core input maps (host-side layout prep, all small
    except seq_feats transpose)."""
    ind = np.zeros((C, C * P), dtype=BF16)
    for k in range(C):
        ind[k, k * P:(k + 1) * P] = 1.0

    common = {
        "wswT": np.ascontiguousarray(Ws_w.T).astype(BF16),
        "wewT": np.ascontiguousarray(We_w.T).astype(BF16),
        "u": np.ascontiguousarray(U.reshape(H, C * H)).astype(BF16),
        "wsb": np.ascontiguousarray(Ws_b.reshape(KH, P, 1)).astype(np.float32),
        "web": np.ascontiguousarray(We_b.reshape(KH, P, 1)).astype(np.float32),
        "wstl": np.ascontiguousarray(W_w[:, :H].T).astype(BF16),
        "wetl": np.ascontiguousarray(W_w[:, H:].T).astype(BF16),
        "wb": np.ascontiguousarray(W_b.reshape(C, 1)).astype(np.float32),
        "ind": ind,
    }
    in_maps = []
    for b in range(B):
        m = dict(common)
        m["xT"] = np.ascontiguousarray(seq_feats[b].T).astype(BF16)
        in_maps.append(m)
    return in_maps


def kernel(seq_feats, U, W_w, W_b, Ws_w, Ws_b, We_w, We_b):
    from concourse.bass_utils import run_bass_kernel_spmd

    seq_feats = np.asarray(seq_feats, dtype=np.float32)
    U = np.asarray(U, dtype=np.float32)
    W_w = np.asarray(W_w, dtype=np.float32)
    W_b = np.asarray(W_b, dtype=np.float32)
    Ws_w = np.asarray(Ws_w, dtype=np.float32)
    Ws_b = np.asarray(Ws_b, dtype=np.float32)
    We_w = np.asarray(We_w, dtype=np.float32)
    We_b = np.asarray(We_b, dtype=np.float32)

    if "nc" not in _CACHE:
        _CACHE["nc"] = _build_nc()
    nc = _CACHE["nc"]

    in_maps = _host_prep(seq_feats, U, W_w, W_b, Ws_w, Ws_b, We_w, We_b)
    res = run_bass_kernel_spmd(nc, in_maps, core_ids=list(range(N_CORES)))
    _CACHE["last_result"] = res

    out = np.stack([res.results[b]["out"] for b in range(B)])  # [B, C, S, S]
    return np.ascontiguousarray(out.transpose(0, 2, 3, 1)).astype(np.float32)
